# revision 1
# baseline (speedup 1.0000x reference)
"""FAVOR+ (Performer) attention kernel for 8 Trainium2 NeuronCores.

Problem: B=4, N=4096, D=512, H=8, DK=64, M=128 (nb_features=256), fp32.

Sharding: 8 cores = 4 batches x 2 head-groups (4 heads each). Each core
computes, for its (batch, 4-head) shard, the full FAVOR pipeline:

  qkv projection -> phi features -> kv = phi(K)^T V (global token sum)
  -> num = phi(Q) kv, den = phi(Q) ksum -> out = (num/den) @ Wout-slice

and writes a feature-major partial output yT (512, 4096).  The host sums
the two head-group partials per batch and transposes back to (N, D).

Layout strategy on-chip:
  * activations stay feature-major (features on partitions, tokens on
    the free dim) for all weight matmuls: stationary weights, moving
    token blocks of 512
  * the k-side phi (which needs per-token absmax/ssq reductions and a
    per-token exp bias) is computed token-major; proj_k comes out
    token-major directly by using kT chunks as the stationary operand
    against a block-diagonal omega rhs
  * kv (+ksum as a 65th column of v_aug) accumulates in PSUM over all
    32 token chunks, then is PE-transposed once (tiny) back into
    feature-major for the q-side contraction
  * the q-side phi needs no shift/norm: the per-token q prefactor
    cancels in num/den, so phi(Q) ~ exp(+-proj_q/dk^0.25) elementwise
  * 1/den is applied via a K=1 broadcast matmul + one DVE multiply per
    head per 512-token block
"""

import contextlib
import sys

if "/opt/trn_rl_repo" not in sys.path:
    sys.path.insert(0, "/opt/trn_rl_repo")

import numpy as np

import concourse.bass as bass
import concourse.tile as tile
from concourse import library_config, mybir

B, N, D = 4, 4096, 512
H, DK = 8, 64
M = 128
NB = 2 * M
F32 = mybir.dt.float32

INV_DKRT = float(1.0 / (DK ** 0.25))
LN_SQRT_NB = float(np.log(np.sqrt(NB)))      # ln 16
SSQ_C = float(1.0 / (2.0 * np.sqrt(DK)))     # ssq_k -> 0.5*||x32||^2
EPS = 1e-6
EPS_LN_C = float(LN_SQRT_NB + np.log(EPS))   # ln(16) + ln(eps)

F32R = mybir.dt.float32r

TOK_CH = N // 128   # 32 token chunks of 128
TOK_B = N // 512    # 8 token blocks of 512


def _split_waits(nc, maxw=1):
    """walrus in this container allows a single embedded sem wait per
    instruction; the Tile exit drain carries several.  Hoist extras onto
    preceding NoOps on the same engine."""
    for _bbname, bb in nc.bb_map.items():
        insts = bb.bb.instructions
        out = []
        for inst in insts:
            si = inst.sync_info
            if si and si.on_wait and len(si.on_wait) > maxw:
                waits = list(si.on_wait)
                k = 0
                while len(waits) > maxw:
                    chunk, waits = waits[:maxw], waits[maxw:]
                    nop = mybir.InstNoOp(
                        name=f"{inst.name}-wsplit{k}", ins=[], outs=[]
                    )
                    k += 1
                    nop.engine = inst.engine
                    nop.sync_info = mybir.SyncInfo(on_wait=chunk, on_update=[])
                    out.append(nop)
                inst.sync_info = mybir.SyncInfo(
                    on_wait=waits, on_update=list(si.on_update or [])
                )
            out.append(inst)
        insts[:] = out


def build_program(use_bv=False, use_bout=False, use_mask=False, split=True, phases="ab"):

    nc = bass.Bass()

    xT = nc.declare_dram_parameter("xT", (D, N), F32R, isOutput=False)
    wqk = nc.declare_dram_parameter("wqk", (D, 512), F32R, isOutput=False)
    wv_d = nc.declare_dram_parameter("wv", (D, 256), F32R, isOutput=False)
    womq = nc.declare_dram_parameter("womq", (128, 512), F32R, isOutput=False)
    womk = nc.declare_dram_parameter("womk", (128, 512), F32R, isOutput=False)
    wy_d = nc.declare_dram_parameter("wy", (256, 512), F32R, isOutput=False)
    bqk_d = nc.declare_dram_parameter("bqk", (128, 4), F32, isOutput=False)
    consts = nc.declare_dram_parameter("consts", (128, 130), F32, isOutput=False)
    ones1_d = nc.declare_dram_parameter("ones1", (1, 512), F32R, isOutput=False)
    # consts columns: [0:128] identity, [128:130] ones_blk
    if use_bv:
        bv_d = nc.declare_dram_parameter("bv", (1, 256), F32R, isOutput=False)
    if use_bout:
        bout_d = nc.declare_dram_parameter("bout", (1, 512), F32R, isOutput=False)
    if use_mask:
        valid_d = nc.declare_dram_parameter(
            "valid", (128, TOK_CH), F32, isOutput=False
        )
    yT = nc.declare_dram_parameter("yT", (D, N), F32, isOutput=True)

    with tile.TileContext(nc) as tc, contextlib.ExitStack() as ctx:
        wpool = ctx.enter_context(tc.tile_pool(name="weights", bufs=1))
        qkpool = ctx.enter_context(tc.tile_pool(name="qk", bufs=1))
        kvtp = ctx.enter_context(tc.tile_pool(name="kvT", bufs=1))

        # ---- constants / weights ------------------------------------
        t_wqk = [wpool.tile([128, 512], F32R, tag=f"wqk{k}", name=f"wqk{k}") for k in range(4)]
        t_wv = [wpool.tile([128, 256], F32R, tag=f"wv{k}", name=f"wv{k}") for k in range(4)]
        for k in range(4):
            nc.sync.dma_start(out=t_wqk[k], in_=wqk[128 * k:128 * (k + 1), :])
            nc.sync.dma_start(out=t_wv[k], in_=wv_d[128 * k:128 * (k + 1), :])
        t_womq = wpool.tile([128, 512], F32R, tag="womq", name="womq")
        nc.sync.dma_start(out=t_womq, in_=womq[:, :])
        t_womk = wpool.tile([128, 512], F32R, tag="womk", name="womk")
        nc.sync.dma_start(out=t_womk, in_=womk[:, :])
        t_wy = [wpool.tile([128, 512], F32R, tag=f"wy{k}", name=f"wy{k}") for k in range(2)]
        for k in range(2):
            nc.sync.dma_start(out=t_wy[k], in_=wy_d[128 * k:128 * (k + 1), :])
        t_bqk = wpool.tile([128, 4], F32, tag="bqk", name="bqk")
        nc.sync.dma_start(out=t_bqk, in_=bqk_d[:, :])
        t_consts = wpool.tile([128, 130], F32, tag="consts", name="consts")
        nc.sync.dma_start(out=t_consts, in_=consts[:, :])
        ident = t_consts[:, 0:128]
        ones_blk = t_consts[:, 128:130]
        t_ones1 = wpool.tile([1, 512], F32R, tag="ones1", name="ones1")
        nc.sync.dma_start(out=t_ones1, in_=ones1_d[:, :])
        if use_bv:
            t_bv = wpool.tile([1, 256], F32R, tag="bv", name="bv")
            nc.sync.dma_start(out=t_bv, in_=bv_d[:, :])
        if use_bout:
            t_bout = wpool.tile([1, 512], F32R, tag="bout", name="bout")
            nc.sync.dma_start(out=t_bout, in_=bout_d[:, :])
        if use_mask:
            t_valid = wpool.tile([128, TOK_CH], F32, tag="valid", name="valid")
            nc.sync.dma_start(out=t_valid, in_=valid_d[:, :])

        # qk[m]: feature-major qkT; m=0,1 -> q heads (0,1),(2,3);
        # m=2,3 -> k heads (0,1),(2,3)
        t_qk = [qkpool.tile([128, N], F32R, tag=f"qk{m}", name=f"qk{m}") for m in range(4)]
        # transposed kv (+ksum col 64) per head, filled in phase A
        t_kvT = [kvtp.tile([128, 2, 65], F32R, tag=f"kvT{h}", name=f"kvT{h}") for h in range(4)]
        # eps/c_q per (token, head): head h row lives at partition 32h
        t_Erows = kvtp.tile([128, N], F32, tag="Erows", name="Erows")
        t_E4 = kvtp.tile([4, N], F32, tag="E4", name="E4")
        dr_E = ctx.enter_context(tc.tile_pool(name="drE", bufs=1, space="DRAM"))

        # ---- S1a + phase A ------------------------------------------
        with tc.tile_pool(name="xt", bufs=1) as xtp, \
             tc.tile_pool(name="worka", bufs=2) as wka, \
             tc.tile_pool(name="psKV", bufs=1, space="PSUM") as psKV:

            t_xt = [xtp.tile([128, N], F32R, tag=f"xt{k}", name=f"xt{k}") for k in range(4)]
            for k in range(4):
                nc.sync.dma_start(out=t_xt[k], in_=xT[128 * k:128 * (k + 1), :])

            t_kv = [psKV.tile([65, 256], F32, tag=f"kv{h}", name=f"kv{h}") for h in range(4)]
            va_bufs = [wka.tile([128, 4, 65], F32R, tag=f"va{i}", name=f"va{i}", bufs=1)
                       for i in range(2)]
            for i in range(2):
                nc.sync.dma_start(
                    out=va_bufs[i][:, :, 64:65],
                    in_=ones1_d[0:1, 0:4].to_broadcast((128, 4)),
                )

            with tc.tile_pool(name="psA", bufs=2, space="PSUM") as psA:
                # S1a: qkT = (wqk chunk)^T @ xT, feature-major
                for m in range(4):
                    for t8 in range(TOK_B):
                        sl = slice(512 * t8, 512 * (t8 + 1))
                        ps = psA.tile([128, 512], F32, tag="pk", name="pk")
                        for k in range(4):
                            nc.tensor.matmul(
                                ps,
                                lhsT=t_wqk[k][:, 128 * m:128 * (m + 1)],
                                rhs=t_xt[k][:, sl],
                                start=(k == 0),
                                stop=(k == 3),
                            )
                        nc.scalar.activation(
                            out=t_qk[m][:, sl], in_=ps,
                            func=mybir.ActivationFunctionType.Identity,
                            bias=t_bqk[:, m:m + 1], scale=1.0,
                        )

                for t in range(TOK_CH):
                    cl = slice(128 * t, 128 * (t + 1))
                    # v chunk token-major (cols 0:256); ssq_k in 256:260
                    pv = psA.tile([128, 392], F32, tag="pv", name="pv")
                    for k in range(4):
                        nc.tensor.matmul(
                            pv[:, 0:256],
                            lhsT=t_xt[k][:, cl], rhs=t_wv[k],
                            start=(k == 0), stop=(k == 3) and not use_bv,
                        )
                    if use_bv:
                        nc.tensor.matmul(
                            pv[:, 0:256],
                            lhsT=t_ones1[:, 0:128], rhs=t_bv,
                            start=False, stop=True,
                        )
                    # proj_k token-major via blockdiag omega
                    pk = psA.tile([128, 512], F32, tag="pk", name="pk")
                    for p in range(2):
                        nc.tensor.matmul(
                            pk[:, 256 * p:256 * (p + 1)],
                            lhsT=t_qk[2 + p][:, cl],
                            rhs=t_womk[:, 256 * p:256 * (p + 1)],
                            start=True, stop=True,
                        )
                    # ssq_k via ones-matmul on squared kT chunk
                    ksqc = wka.tile([128, 2, 128], F32, tag="ksqc", name="ksqc")
                    for p in range(2):
                        nc.scalar.square(ksqc[:, p, :], t_qk[2 + p][:, cl])
                    for p in range(2):
                        nc.tensor.matmul(
                            pv[:, 256 + 2 * p:258 + 2 * p],
                            lhsT=ksqc[:, p, :], rhs=ones_blk,
                            start=True, stop=True, skip_group_check=True,
                        )
                    # q-side proj (token-major) for the eps correction
                    pq2 = psA.tile([128, 512], F32, tag="pk", name="pq2")
                    for p in range(2):
                        nc.tensor.matmul(
                            pq2[:, 256 * p:256 * (p + 1)],
                            lhsT=t_qk[p][:, cl],
                            rhs=t_womk[:, 256 * p:256 * (p + 1)],
                            start=True, stop=True,
                        )
                    qsqc = wka.tile([128, 2, 128], F32, tag="ksqc", name="qsqc")
                    for p in range(2):
                        nc.scalar.square(qsqc[:, p, :], t_qk[p][:, cl])
                    for p in range(2):
                        nc.tensor.matmul(
                            pv[:, 260 + 2 * p:262 + 2 * p],
                            lhsT=qsqc[:, p, :], rhs=ones_blk,
                            start=True, stop=True, skip_group_check=True,
                        )
                    srdq = wka.tile([128, 4], F32, tag="srdq", name="srdq")
                    nc.vector.tensor_reduce(
                        out=srdq,
                        in_=pq2.rearrange("p (h m) -> p h m", h=4),
                        axis=mybir.AxisListType.X,
                        op=mybir.AluOpType.max,
                        apply_absolute_value=True,
                    )
                    ssqqs = wka.tile([128, 4], F32, tag="ssqqs", name="ssqqs")
                    nc.vector.tensor_scalar(
                        out=ssqqs, in0=pv[:, 260:264],
                        scalar1=SSQ_C, scalar2=EPS_LN_C,
                        op0=mybir.AluOpType.mult, op1=mybir.AluOpType.add,
                    )
                    comb = wka.tile([128, 4], F32, tag="comb", name="comb")
                    nc.vector.scalar_tensor_tensor(
                        out=comb, in0=srdq, scalar=INV_DKRT, in1=ssqqs,
                        op0=mybir.AluOpType.mult, op1=mybir.AluOpType.add,
                    )
                    ecp = wka.tile([128, 97], F32, tag="ecp", name="ecp")
                    nc.vector.memset(ecp, 0.0)
                    for h in range(4):
                        nc.scalar.activation(
                            out=ecp[:, 32 * h:32 * h + 1],
                            in_=comb[:, h:h + 1],
                            func=mybir.ActivationFunctionType.Exp,
                            bias=0.0, scale=1.0,
                        )
                    nc.tensor.transpose(pv[0:97, 264:392], ecp, ident)
                    nc.vector.tensor_copy(
                        out=t_Erows[0:97, cl], in_=pv[0:97, 264:392]
                    )
                    # shift_k = absmax over m (free dim), per head
                    srd = wka.tile([128, 4], F32, tag="srd", name="srd")
                    nc.vector.tensor_reduce(
                        out=srd,
                        in_=pk.rearrange("p (h m) -> p h m", h=4),
                        axis=mybir.AxisListType.X,
                        op=mybir.AluOpType.max,
                        apply_absolute_value=True,
                    )
                    # bias_k = -(srd/dkrt + ssq/(2 sqrt(dk)) + ln 16)
                    ssqs = wka.tile([128, 4], F32, tag="ssqs", name="ssqs")
                    nc.vector.tensor_scalar(
                        out=ssqs, in0=pv[:, 256:260],
                        scalar1=SSQ_C, scalar2=LN_SQRT_NB,
                        op0=mybir.AluOpType.mult, op1=mybir.AluOpType.add,
                    )
                    bk = wka.tile([128, 4], F32, tag="bk", name="bk")
                    nc.vector.scalar_tensor_tensor(
                        out=bk, in0=srd, scalar=-INV_DKRT, in1=ssqs,
                        op0=mybir.AluOpType.mult, op1=mybir.AluOpType.subtract,
                    )
                    # v_aug: [v_h | 1]
                    va = va_bufs[t % 2]
                    nc.vector.tensor_copy(
                        out=va[:, :, 0:64],
                        in_=pv[:, 0:256].rearrange("p (h d) -> p h d", h=4),
                    )
                    # k_phi = exp(+-pk/dkrt + bias_k), token-major
                    kph = wka.tile([128, 4, 256], F32R, tag="kph", name="kph")
                    for h in range(4):
                        hs = slice(128 * h, 128 * (h + 1))
                        nc.scalar.activation(
                            out=kph[:, h, 0:128], in_=pk[:, hs],
                            func=mybir.ActivationFunctionType.Exp,
                            bias=bk[:, h:h + 1], scale=INV_DKRT,
                        )
                        nc.scalar.activation(
                            out=kph[:, h, 128:256], in_=pk[:, hs],
                            func=mybir.ActivationFunctionType.Exp,
                            bias=bk[:, h:h + 1], scale=-INV_DKRT,
                        )
                    if use_mask:
                        nc.vector.tensor_scalar_mul(
                            kph.rearrange("p h f -> p (h f)"),
                            kph.rearrange("p h f -> p (h f)"),
                            t_valid[:, t:t + 1],
                        )
                    # kv (+ksum row 64) accumulation over token chunks
                    for h in range(4):
                        nc.tensor.matmul(
                            t_kv[h],
                            lhsT=va[:, h, :], rhs=kph[:, h, :],
                            start=(t == 0), stop=(t == TOK_CH - 1),
                            skip_group_check=True,
                        )

            # transpose kv_aug -> feature-major kvT (psA closed: banks free)
            with tc.tile_pool(name="psT", bufs=2, space="PSUM") as psT:
                for h in range(4):
                    tmp = wka.tile([65, 256], F32, tag="kvtmp", name="kvtmp")
                    nc.vector.tensor_copy(out=tmp, in_=t_kv[h])
                    for j in range(2):
                        pt = psT.tile([128, 65], F32, tag="pt", name="pt")
                        nc.tensor.transpose(
                            pt, tmp[:, 128 * j:128 * (j + 1)],
                            ident[0:65, 0:65],
                        )
                        nc.vector.tensor_copy(out=t_kvT[h][:, j, :], in_=pt)
                edr = dr_E.tile([4, N], F32, tag="edr", name="edr")
                for h in range(4):
                    nc.sync.dma_start(
                        out=edr[h:h + 1, :], in_=t_Erows[32 * h:32 * h + 1, :]
                    )
                nc.sync.dma_start(out=t_E4, in_=edr[:, :])

        if "b" not in phases:
            with tc.tile_pool(name="dbg", bufs=1) as dbgp:
                nc.sync.dma_start(out=yT[0:128, :], in_=t_Erows)
                for h in range(4):
                    dk_t = dbgp.tile([128, 130], F32, tag=f"dbgkv{h}", name=f"dbgkv{h}")
                    nc.vector.tensor_copy(
                        out=dk_t, in_=t_kvT[h].rearrange("p a b -> p (a b)"))
                    nc.sync.dma_start(
                        out=yT[128:256, 130 * h:130 * (h + 1)], in_=dk_t)
            _phB = False
        else:
            _phB = True
        # ---- phase B ------------------------------------------------
        with tc.tile_pool(name="workb", bufs=2) as wkb, \
             tc.tile_pool(name="drb", bufs=2, space="DRAM") as drb, \
             tc.tile_pool(name="psB", bufs=2, space="PSUM") as psB, \
             tc.tile_pool(name="psY", bufs=2, space="PSUM") as psY:
            for t8 in range(TOK_B if _phB else 0):
                sl = slice(512 * t8, 512 * (t8 + 1))
                ns = [wkb.tile([128, 512], F32R, tag=f"ns{d}", name=f"ns{d}") for d in range(2)]
                pns = []
                dsbs = []
                if phases == "b2":
                    for d in range(2):
                        nc.vector.memset(ns[d], 0.5)
                for h in range(4 if phases != "b2" else 0):
                    pq = psB.tile([128, 512], F32, tag="pq", name="pq", bufs=1)
                    nc.tensor.matmul(
                        pq,
                        lhsT=t_womq[:, 128 * h:128 * (h + 1)],
                        rhs=t_qk[h // 2][:, sl],
                        start=True, stop=True,
                    )
                    qp = wkb.tile([128, 2, 512], F32R, tag="qp", name="qp")
                    nc.scalar.activation(
                        out=qp[:, 0, :], in_=pq,
                        func=mybir.ActivationFunctionType.Exp,
                        bias=0.0, scale=INV_DKRT,
                    )
                    nc.scalar.activation(
                        out=qp[:, 1, :], in_=pq,
                        func=mybir.ActivationFunctionType.Exp,
                        bias=0.0, scale=-INV_DKRT,
                    )
                    pn = psB.tile([65, 512], F32, tag="pn", name="pn", bufs=4)
                    for j in range(2):
                        nc.tensor.matmul(
                            pn,
                            lhsT=t_kvT[h][:, j, :], rhs=qp[:, j, :],
                            start=(j == 0), stop=(j == 1),
                        )
                    dsb = wkb.tile([1, 512], F32, tag=f"dsb{h}", name=f"dsb{h}")
                    nc.scalar.copy(out=dsb, in_=pn[64:65, :])
                    dsbs.append(dsb)
                    pns.append(pn)
                ddr = drb.tile([4, 512], F32, tag="ddr", name="ddr")
                for h in range(4):
                    nc.sync.dma_start(out=ddr[h:h + 1, :], in_=dsbs[h])
                den4 = wkb.tile([4, 512], F32, tag="den4", name="den4")
                nc.sync.dma_start(out=den4, in_=ddr[:, :])
                nc.vector.tensor_tensor(
                    out=den4, in0=den4, in1=t_E4[:, sl],
                    op=mybir.AluOpType.add,
                )
                nc.vector.reciprocal(out=den4, in_=den4)
                dr = drb.tile([4, 512], F32, tag="dr", name="dr")
                nc.sync.dma_start(out=dr[:, :], in_=den4)
                for h in range(4):
                    pbs = wkb.tile([64, 512], F32, tag="pbs", name="pbs")
                    nc.sync.dma_start(
                        out=pbs,
                        in_=dr[h:h + 1, :].to_broadcast((64, 512)),
                    )
                    nc.vector.tensor_tensor(
                        out=ns[h // 2][64 * (h % 2):64 * (h % 2) + 64, :],
                        in0=pns[h][0:64, :], in1=pbs,
                        op=mybir.AluOpType.mult,
                    )
                for m4 in range(4):
                    py = psY.tile([128, 512], F32, tag="py", name="py")
                    for d in range(2):
                        nc.tensor.matmul(
                            py,
                            lhsT=t_wy[d][:, 128 * m4:128 * (m4 + 1)],
                            rhs=ns[d],
                            start=(d == 0),
                            stop=(d == 1) and not use_bout,
                        )
                    if use_bout:
                        nc.tensor.matmul(
                            py,
                            lhsT=t_bout[0:1, 128 * m4:128 * (m4 + 1)],
                            rhs=t_ones1[:, 0:512],
                            start=False, stop=True,
                        )
                    ysb = wkb.tile([128, 512], F32, tag="ysb", name="ysb")
                    nc.vector.tensor_copy(out=ysb, in_=py)
                    nc.sync.dma_start(
                        out=yT[128 * m4:128 * (m4 + 1), sl], in_=ysb,
                    )

    if split:
        _split_waits(nc)
    return nc


_PROGRAM_CACHE = {}


def _get_program(use_bv, use_bout, use_mask):
    key = (use_bv, use_bout, use_mask)
    if key not in _PROGRAM_CACHE:
        _PROGRAM_CACHE[key] = build_program(*key)
    return _PROGRAM_CACHE[key]


def make_in_maps(x, key_padding_mask, Wqkv, bqkv, Wout, bout, omega):
    """Shard + lay out the full inputs into 8 per-core input maps."""
    Wq, Wk, Wv = Wqkv[0:D], Wqkv[D:2 * D], Wqkv[2 * D:3 * D]
    bq, bk_, bv = bqkv[0:D], bqkv[D:2 * D], bqkv[2 * D:3 * D]
    mask = key_padding_mask

    use_bv = bool(np.any(bv != 0))
    use_bout = bool(np.any(bout != 0))
    use_mask = bool(np.any(mask))

    consts = np.zeros((128, 130), np.float32)
    consts[:, 0:128] = np.eye(128, dtype=np.float32)
    consts[0:64, 128] = 1.0
    consts[64:128, 129] = 1.0

    in_maps = []
    for c in range(8):
        b, hg = c // 2, c % 2
        dsl = slice(256 * hg, 256 * (hg + 1))
        heads = [4 * hg + i for i in range(4)]
        wqk_c = np.concatenate([Wq.T[:, dsl], Wk.T[:, dsl]], axis=1)
        womq_c = np.zeros((128, 512), np.float32)
        womk_c = np.zeros((128, 512), np.float32)
        for i, g in enumerate(heads):
            off = 64 * (i % 2)
            womq_c[off:off + 64, 128 * i:128 * (i + 1)] = omega[g].T
        for p in range(2):
            womk_c[0:64, 256 * p:256 * p + 128] = omega[heads[2 * p]].T
            womk_c[64:128, 256 * p + 128:256 * p + 256] = omega[heads[2 * p + 1]].T
        bqk_vec = np.concatenate([bq[dsl], bk_[dsl]])
        im = {
            "xT": np.ascontiguousarray(x[b].T),
            "wqk": np.ascontiguousarray(wqk_c),
            "wv": np.ascontiguousarray(Wv.T[:, dsl]),
            "womq": womq_c,
            "womk": womk_c,
            "wy": np.ascontiguousarray(Wout[:, dsl].T),
            "bqk": np.ascontiguousarray(bqk_vec.reshape(4, 128).T),
            "consts": consts,
            "ones1": np.ones((1, 512), np.float32),
        }
        if use_bv:
            im["bv"] = np.ascontiguousarray(bv[None, :])
        if use_bout:
            im["bout"] = np.ascontiguousarray(
                (bout if hg == 0 else np.zeros_like(bout))[None, :]
            )
        if use_mask:
            im["valid"] = np.ascontiguousarray(
                (~mask[b]).astype(np.float32).reshape(TOK_CH, 128).T
            )
        in_maps.append(im)
    return in_maps, (use_bv, use_bout, use_mask)


def gather_output(per_core_yT):
    """Sum head-group partials and transpose back to (B, N, D)."""
    y = np.empty((B, N, D), np.float32)
    for b in range(B):
        acc = per_core_yT[2 * b] + per_core_yT[2 * b + 1]
        y[b] = acc.T
    return y


def kernel(x, key_padding_mask, Wqkv, bqkv, Wout, bout, omega):
    from concourse.bass_utils import run_bass_kernel_spmd

    x = np.asarray(x, np.float32)
    mask = np.asarray(key_padding_mask)
    Wqkv = np.asarray(Wqkv, np.float32)
    bqkv = np.asarray(bqkv, np.float32)
    Wout = np.asarray(Wout, np.float32)
    bout = np.asarray(bout, np.float32)
    omega = np.asarray(omega, np.float32)

    in_maps, flags = make_in_maps(x, mask, Wqkv, bqkv, Wout, bout, omega)
    nc = _get_program(*flags)
    res = run_bass_kernel_spmd(nc, in_maps, list(range(8)))
    return gather_output([r["yT"] for r in res.results])



# revision 42
# speedup vs baseline: 2.3422x; 2.3422x over previous
"""FAVOR+ (Performer) attention kernel for 8 Trainium2 NeuronCores.

Problem: B=4, N=4096, D=512, H=8, DK=64, M=128 (nb_features=256), fp32.

Sharding: 8 cores = 4 batches x 2 head-groups (4 heads each).  Each core
computes, for its (batch, 4-head) shard, the full FAVOR pipeline and
writes a feature-major partial output yT (512, 4096); the host sums the
two head-group partials per batch, adds bout, and transposes.

Math simplifications vs the reference (validated numerically, rel err
~8e-3 vs fp64 reference, tolerance 2e-2):
  * the EPS=1e-6 den-regularizer is dropped (contributes <= ~5e-3)
  * the q-side per-token prefactor cancels in num/den, so
    phi_q ~ exp(+-proj_q) with no shift/norm
  * the k-side per-token factor c_k = exp(-shift_k - ssq_k/(2 sqrt dk)
    - ln sqrt(2M)) is folded into the v vectors (and a ksum column)
    instead of the exponent bias, so phi_k = exp(+-proj_k) needs no bias
  * x, W, q, k, sq(k), phi_k, v*c_k are bf16 (halves DMA + LDWEIGHTS and
    enables fast-weight-load); everything else fp32

Engine balance per core (est): PE ~50us matmul stream; ACT ~80us of
exps in [128,1024] batches; DVE ~70us of reduces/drains/divides; GPSIMD
squares + den partition-broadcasts; DMA ~12MB fully overlapped.

Layouts:
  * S1: q,k feature-major tiles [128, 4096] (bf16) via stationary W
    chunks; k tiles squared on GPSIMD into sq tiles for the ssq matmul
  * phase A (per 128-token chunk): v token-major [128tok, (4h,64d)]
    (stationary xT chunk), proj_k token-major [128tok, (4h,128m)],
    ssq via ones-indicator matmul appended to the pv bank, shift_k via
    DVE abs-max reduce, c_k = exp(...) on ACT, va = [ck | ck*v] bf16,
    kv accumulated FEATURE-major: lhsT=phi_k chunk [128tok,128m],
    rhs=va [128tok, 65] -> kvT[m, (ksum, 64d)] in PSUM over 32 chunks
  * phase B1 (per head, token-block pair): proj_q feature-major
    (stationary womq), qp = exp(+-proj_q) fp32, pn[65,512] = kvT @ qp
    with row 0 = den; reciprocal_approx_fast -> partition_broadcast ->
    DVE multiply into ns tiles [128, 8, 512]
  * phase B2: y = Wout-slice @ ns, 8-bank PSUM sweeps, drains split
    ACT/DVE, DMA out
"""

import contextlib
import sys

if "/opt/trn_rl_repo" not in sys.path:
    sys.path.insert(0, "/opt/trn_rl_repo")

import numpy as np

import concourse.bass as bass
import concourse.tile as tile
from concourse import library_config, mybir

B, N, D = 4, 4096, 512
H, DK = 8, 64
M = 128
NB = 2 * M
F32 = mybir.dt.float32
F32R = mybir.dt.float32r
BF16 = mybir.dt.bfloat16

INV_DKRT = float(1.0 / (DK ** 0.25))
LN_SQRT_NB = float(np.log(np.sqrt(NB)))      # ln 16
SSQ_C = float(1.0 / (2.0 * np.sqrt(DK)))     # ssq_k -> 0.5*||x32||^2
EXP_SHIFT = 16.0                             # static stabilizer, > max|proj|

TOK_CH = N // 128   # 32 token chunks of 128
TOK_B = N // 512    # 8 token blocks of 512
NPAIR = TOK_CH // 2  # 16 chunk pairs


def _split_waits(nc, maxw=1):
    """walrus in this container allows a single embedded sem wait per
    instruction; hoist extras onto preceding NoOps on the same engine."""
    for _bbname, bb in nc.bb_map.items():
        insts = bb.bb.instructions
        out = []
        for inst in insts:
            si = inst.sync_info
            if si and si.on_wait and len(si.on_wait) > maxw:
                waits = list(si.on_wait)
                k = 0
                while len(waits) > maxw:
                    chunk, waits = waits[:maxw], waits[maxw:]
                    nop = mybir.InstNoOp(
                        name=f"{inst.name}-wsplit{k}", ins=[], outs=[]
                    )
                    k += 1
                    nop.engine = inst.engine
                    nop.sync_info = mybir.SyncInfo(on_wait=chunk, on_update=[])
                    out.append(nop)
                inst.sync_info = mybir.SyncInfo(
                    on_wait=waits, on_update=list(si.on_update or [])
                )
            out.append(inst)
        insts[:] = out


def build_program(use_bv=False, use_mask=False, split=True, debug=False):
    nc = bass.Bass()
    AF = mybir.ActivationFunctionType
    if debug:
        dbg_d = nc.declare_dram_parameter("dbg", (128, 4096), F32, isOutput=True)

    xT = nc.declare_dram_parameter("xT", (D, N), BF16, isOutput=False)
    wqk_d = nc.declare_dram_parameter("wqk", (D, 512), BF16, isOutput=False)
    wv_d = nc.declare_dram_parameter("wv", (D, 256), BF16, isOutput=False)
    womk_d = nc.declare_dram_parameter("womk", (128, 512), BF16, isOutput=False)
    womq_d = nc.declare_dram_parameter("womq", (128, 512), BF16, isOutput=False)
    wy_d = nc.declare_dram_parameter("wy", (256, 512), F32R, isOutput=False)
    bqk_d = nc.declare_dram_parameter("bqk", (128, 4), F32, isOutput=False)
    onesi_d = nc.declare_dram_parameter("onesi", (128, 2), BF16, isOutput=False)
    ind4_d = nc.declare_dram_parameter("ind4", (97, 256), F32R, isOutput=False)
    if use_bv:
        bvb_d = nc.declare_dram_parameter("bvb", (128, 256), F32, isOutput=False)
    if use_mask:
        valid_d = nc.declare_dram_parameter(
            "valid", (128, TOK_CH), F32, isOutput=False
        )
    yT = nc.declare_dram_parameter("yT", (D, N), F32, isOutput=True)

    with tile.TileContext(nc) as tc, contextlib.ExitStack() as ctx:
        wpool = ctx.enter_context(tc.tile_pool(name="weights", bufs=1))
        big = ctx.enter_context(tc.tile_pool(name="big", bufs=1))

        # ---- weights ------------------------------------------------
        t_wqk = [wpool.tile([128, 512], BF16, tag=f"wqk{k}", name=f"wqk{k}")
                 for k in range(4)]
        t_wv = [wpool.tile([128, 256], BF16, tag=f"wv{k}", name=f"wv{k}")
                for k in range(4)]
        for k in range(4):
            nc.sync.dma_start(out=t_wqk[k], in_=wqk_d[128 * k:128 * (k + 1), :])
            nc.sync.dma_start(out=t_wv[k], in_=wv_d[128 * k:128 * (k + 1), :])
        t_womk = wpool.tile([128, 512], BF16, tag="womk", name="womk")
        nc.sync.dma_start(out=t_womk, in_=womk_d[:, :])
        t_womq = wpool.tile([128, 512], BF16, tag="womq", name="womq")
        nc.sync.dma_start(out=t_womq, in_=womq_d[:, :])
        t_wy = [wpool.tile([128, 512], F32R, tag=f"wy{k}", name=f"wy{k}")
                for k in range(2)]
        for k in range(2):
            nc.sync.dma_start(out=t_wy[k], in_=wy_d[128 * k:128 * (k + 1), :])
        t_bqk = wpool.tile([128, 4], F32, tag="bqk", name="bqk")
        nc.sync.dma_start(out=t_bqk, in_=bqk_d[:, :])
        t_onesi = wpool.tile([128, 2], BF16, tag="onesi", name="onesi")
        nc.sync.dma_start(out=t_onesi, in_=onesi_d[:, :])
        # static exp shift: exp args stay <= 0 (ACT spline accuracy); the
        # q-side factor cancels in num/den, the k-side folds into c_k
        t_b16 = wpool.tile([128, 1], F32, tag="b16", name="b16")
        nc.vector.memset(t_b16, -EXP_SHIFT)
        t_ckb = wpool.tile([128, 1], F32, tag="ckb", name="ckb")
        nc.vector.memset(t_ckb, EXP_SHIFT - LN_SQRT_NB)
        t_ind4 = wpool.tile([97, 256], F32R, tag="ind4", name="ind4")
        nc.sync.dma_start(out=t_ind4, in_=ind4_d[:, :])
        if use_bv:
            t_bvb = wpool.tile([128, 256], F32, tag="bvb", name="bvb")
            nc.sync.dma_start(out=t_bvb, in_=bvb_d[:, :])
        if use_mask:
            t_valid = wpool.tile([128, TOK_CH], F32, tag="valid", name="valid")
            nc.sync.dma_start(out=t_valid, in_=valid_d[:, :])

        # ---- persistent activation tiles ----------------------------
        t_xt = [big.tile([128, N], BF16, tag=f"xt{k}", name=f"xt{k}")
                for k in range(4)]
        for t8 in range(TOK_B):      # block-major so S1 can start early
            sl = slice(512 * t8, 512 * (t8 + 1))
            for k in range(4):
                nc.sync.dma_start(out=t_xt[k][:, sl],
                                  in_=xT[128 * k:128 * (k + 1), sl])
        # m=0,1 -> q heads (0,1),(2,3); m=2,3 -> k
        t_qk = [big.tile([128, N], BF16, tag=f"qk{m}", name=f"qk{m}")
                for m in range(4)]
        t_sq = [big.tile([128, N], BF16, tag=f"sq{p}", name=f"sq{p}")
                for p in range(2)]
        # kvT[s][m, (ksum, 64 d)] per head; s in {+, -}
        t_kvT = big.tile([128, 2, 4, 65], F32R, tag="kvT", name="kvT")
        # ns[d-group][:, t8, :] fp32 for the final projection
        t_ns = [big.tile([128, TOK_B, 512], F32R, tag=f"ns{d}", name=f"ns{d}")
                for d in range(2)]

        def s1_block(psS, m, t8, drain_eng):
            sl = slice(512 * t8, 512 * (t8 + 1))
            ps = psS.tile([128, 512], F32, tag="psS", name=f"psS{m}_{t8}")
            for kk in range(4):
                nc.tensor.matmul(
                    ps,
                    lhsT=t_wqk[kk][:, 128 * m:128 * (m + 1)],
                    rhs=t_xt[kk][:, sl],
                    start=(kk == 0), stop=(kk == 3),
                )
            if drain_eng == "act":
                nc.scalar.activation(
                    out=t_qk[m][:, sl], in_=ps, func=AF.Identity,
                    bias=t_bqk[:, m:m + 1], scale=1.0,
                )
            else:
                nc.vector.tensor_scalar(
                    out=t_qk[m][:, sl], in0=ps,
                    scalar1=t_bqk[:, m:m + 1], scalar2=None,
                    op0=mybir.AluOpType.add,
                )
            if m >= 2:
                nc.scalar.activation(
                    out=t_sq[m - 2][:, sl], in_=t_qk[m][:, sl],
                    func=AF.Square, bias=0.0, scale=1.0,
                )

        # ---- S1-k: k feature-major tiles (m=2,3) --------------------
        with tc.tile_pool(name="psSk", bufs=4, space="PSUM") as psSk:
            for m in (2, 3):
                for t8 in range(TOK_B):
                    s1_block(psSk, m, t8, "act" if t8 % 2 == 0 else "dve")

        # ---- phase A + interleaved S1-q -----------------------------
        with tc.tile_pool(name="psK", bufs=1, space="PSUM") as psK, \
             tc.tile_pool(name="psV", bufs=2, space="PSUM") as psV, \
             tc.tile_pool(name="psKV", bufs=1, space="PSUM") as psKV, \
             tc.tile_pool(name="psSq", bufs=2, space="PSUM") as psSq, \
             tc.tile_pool(name="wka", bufs=2) as wka:

            # kv accumulators: [m, (ksum, d0..d63)] per (sign, head)
            kvps = [psKV.tile([128, 4, 65], F32, tag=f"kvp{s}", name=f"kvp{s}")
                    for s in range(2)]

            for p in range(NPAIR):
                c0 = 2 * p
                # proj_k for both chunks of the pair -> [128, 2, 512]
                pk2 = psK.tile([128, 2, 512], F32, tag="pk2", name="pk2")
                for ci in range(2):
                    cl = slice(128 * (c0 + ci), 128 * (c0 + ci) + 128)
                    for pp in range(2):
                        nc.tensor.matmul(
                            pk2[:, ci, 256 * pp:256 * (pp + 1)],
                            lhsT=t_qk[2 + pp][:, cl],
                            rhs=t_womk[:, 256 * pp:256 * (pp + 1)],
                            start=True, stop=True,
                        )
                # phi_k = exp(+-proj_k), bf16, one ACT inst per sign
                kph = wka.tile([128, 2, 2, 512], BF16, tag="kph", name="kph")
                nc.scalar.activation(
                    out=kph[:, 0, :, :], in_=pk2, func=AF.Exp,
                    bias=t_b16[:, 0:1], scale=1.0,
                )
                nc.scalar.activation(
                    out=kph[:, 1, :, :], in_=pk2, func=AF.Exp,
                    bias=t_b16[:, 0:1], scale=-1.0,
                )
                # shift_k = absmax over m per (chunk, head)
                srd = wka.tile([128, 2, 4], F32, tag="srd", name="srd")
                nc.vector.tensor_reduce(
                    out=srd,
                    in_=pk2.rearrange("p c (h m) -> p (c h) m", h=4),
                    axis=mybir.AxisListType.X,
                    op=mybir.AluOpType.max,
                    apply_absolute_value=True,
                )
                ck8 = wka.tile([128, 2, 4], F32, tag="ck8", name="ck8")
                pvs = []
                for ci in range(2):
                    c = c0 + ci
                    cl = slice(128 * c, 128 * c + 128)
                    pv = psV.tile([128, 260], F32, tag="pv", name="pv")
                    pvs.append(pv)
                    for kk in range(4):
                        nc.tensor.matmul(
                            pv[:, 0:256],
                            lhsT=t_xt[kk][:, cl], rhs=t_wv[kk],
                            start=(kk == 0), stop=(kk == 3),
                        )
                    for pp in range(2):
                        nc.tensor.matmul(
                            pv[:, 256 + 2 * pp:258 + 2 * pp],
                            lhsT=t_sq[pp][:, cl], rhs=t_onesi,
                            start=True, stop=True, skip_group_check=True,
                        )
                    # bias = shift + SSQ_C*ssq  (exp(-bias - ln16) = c_k)
                    nc.vector.scalar_tensor_tensor(
                        out=ck8[:, ci, :], in0=pv[:, 256:260],
                        scalar=SSQ_C, in1=srd[:, ci, :],
                        op0=mybir.AluOpType.mult, op1=mybir.AluOpType.add,
                    )
                # c_k for both chunks in one tiny ACT inst
                nc.scalar.activation(
                    out=ck8, in_=ck8, func=AF.Exp,
                    bias=t_ckb[:, 0:1], scale=-1.0,
                )
                if use_mask:
                    for ci in range(2):
                        nc.vector.tensor_scalar(
                            out=ck8[:, ci, :], in0=ck8[:, ci, :],
                            scalar1=t_valid[:, c0 + ci:c0 + ci + 1],
                            scalar2=None, op0=mybir.AluOpType.mult,
                        )
                for ci in range(2):
                    # va = [c_k * v | c_k] per head, bf16
                    va = wka.tile([128, 4, 65], BF16, tag="va", name="va")
                    nc.vector.tensor_copy(out=va[:, :, 64], in_=ck8[:, ci, :])
                    for h in range(4):
                        nc.vector.tensor_scalar(
                            out=va[:, h, 0:64],
                            in0=pvs[ci][:, 64 * h:64 * (h + 1)],
                            scalar1=ck8[:, ci, h:h + 1], scalar2=None,
                            op0=mybir.AluOpType.mult,
                        )
                    # kv accumulation, feature-major
                    # start=True zeroes a whole 2KB bank (clearing sibling
                    # groups' has-written flags), so only the first matmul
                    # per kv bank starts; later groups overwrite-on-first-
                    # touch thanks to the cleared flags, then accumulate.
                    for s in range(2):
                        for h in range(4):
                            nc.tensor.matmul(
                                kvps[s][:, h, :],
                                lhsT=kph[:, s, ci, 128 * h:128 * (h + 1)],
                                rhs=va[:, h, :],
                                start=(c0 + ci == 0 and h == 0),
                                stop=(c0 + ci == TOK_CH - 1),
                                skip_group_check=True,
                            )
                # one interleaved S1-q block per pair: m=0: p even
                s1_block(psSq, p % 2, p // 2, "act" if p % 4 < 2 else "dve")
                if debug and p == NPAIR - 1:
                    dbg_tiles = {"va": va, "ck8": ck8, "kph": kph, "srd": srd}

            if debug:
                dva = wka.tile([128, 260], F32, tag="dva", name="dva")
                nc.vector.tensor_copy(
                    out=dva,
                    in_=dbg_tiles["va"].rearrange("p a b -> p (a b)"))
                nc.sync.dma_start(out=dbg_d[:, 520:780], in_=dva)
                dck = wka.tile([128, 16], F32, tag="dck", name="dck")
                nc.vector.tensor_copy(
                    out=dck[:, 0:8],
                    in_=dbg_tiles["ck8"].rearrange("p a b -> p (a b)"))
                nc.vector.tensor_copy(
                    out=dck[:, 8:16],
                    in_=dbg_tiles["srd"].rearrange("p a b -> p (a b)"))
                nc.sync.dma_start(out=dbg_d[:, 780:796], in_=dck)
                dkp = wka.tile([128, 2048], F32, tag="dkp", name="dkp")
                nc.vector.tensor_copy(
                    out=dkp,
                    in_=dbg_tiles["kph"].rearrange("p a b c -> p (a b c)"))
                nc.sync.dma_start(out=dbg_d[:, 2048:4096], in_=dkp)

            # kvT to SBUF (fp32r) for phase B stationaries
            for s in range(2):
                nc.vector.tensor_copy(out=t_kvT[:, s, :, :], in_=kvps[s])
            if use_bv:
                tmpb = wka.tile([128, 4, 64], F32, tag="tmpb", name="tmpb")
                for s in range(2):
                    for h in range(4):
                        nc.vector.tensor_scalar(
                            out=tmpb[:, h, :], in0=t_bvb[:, 64 * h:64 * (h + 1)],
                            scalar1=t_kvT[:, s, h, 64:65], scalar2=None,
                            op0=mybir.AluOpType.mult,
                        )
                    nc.vector.tensor_tensor(
                        out=t_kvT[:, s, :, 0:64], in0=t_kvT[:, s, :, 0:64],
                        in1=tmpb, op=mybir.AluOpType.add,
                    )

        if debug:
            with tc.tile_pool(name="dbgp", bufs=1) as dbgp:
                dkv = dbgp.tile([128, 520], F32, tag="dkv", name="dkv")
                nc.vector.tensor_copy(
                    out=dkv, in_=t_kvT.rearrange("p a b c -> p (a b c)"))
                nc.sync.dma_start(out=dbg_d[:, 0:520], in_=dkv)
                dqk = dbgp.tile([128, 512], F32, tag="dqk", name="dqk")
                for m in range(4):
                    nc.vector.tensor_copy(out=dqk, in_=t_qk[m][:, 0:512])
                    nc.sync.dma_start(
                        out=dbg_d[:, 1024 + 512 * m:1024 + 512 * (m + 1)],
                        in_=dqk)


        # ---- phase B1: num/den, ns tiles ----------------------------
        # t8-pair outer / head inner; den rows collected at partitions
        # 32h of a [97,512] tile per block, 1/den = exp(-ln(den)) on ACT
        # (both funcs in the natural_log_exp_and_others table set), then
        # per-head PE row-broadcast + DVE multiply into ns.
        with tc.tile_pool(name="psQ", bufs=2, space="PSUM") as psQ, \
             tc.tile_pool(name="psN", bufs=2, space="PSUM") as psN, \
             tc.tile_pool(name="wkb", bufs=2) as wkb:
            for bp in range(TOK_B // 2):
                dsb4 = [wkb.tile([97, 512], F32, tag=f"dsb4_{ci}",
                                 name=f"dsb4_{ci}", bufs=2) for ci in range(2)]
                for ci in range(2):
                    # unused partitions must stay finite through ln/exp
                    nc.vector.memset(dsb4[ci], 1.0)
                pns = {}
                for h in range(4):
                    pq2 = psQ.tile([128, 2, 512], F32, tag="pq2", name="pq2")
                    for ci in range(2):
                        sl = slice(1024 * bp + 512 * ci,
                                   1024 * bp + 512 * (ci + 1))
                        nc.tensor.matmul(
                            pq2[:, ci, :],
                            lhsT=t_womq[:, 128 * h:128 * (h + 1)],
                            rhs=t_qk[h // 2][:, sl],
                            start=True, stop=True,
                        )
                    qp = wkb.tile([128, 2, 2, 512], F32R, tag="qp", name="qp")
                    nc.scalar.activation(
                        out=qp[:, 0, :, :], in_=pq2, func=AF.Exp,
                        bias=t_b16[:, 0:1], scale=1.0,
                    )
                    nc.scalar.activation(
                        out=qp[:, 1, :, :], in_=pq2, func=AF.Exp,
                        bias=t_b16[:, 0:1], scale=-1.0,
                    )
                    for ci in range(2):
                        t8 = 2 * bp + ci
                        pn = psN.tile([65, 512], F32, tag="pn", name="pn",
                                      bufs=2)
                        pns[(h, ci)] = pn
                        for s in range(2):
                            nc.tensor.matmul(
                                pn,
                                lhsT=t_kvT[:, s, h, :],
                                rhs=qp[:, s, ci, :],
                                start=(s == 0), stop=(s == 1),
                            )
                        nsl = t_ns[h // 2][64 * (h % 2):64 * (h % 2) + 64,
                                           t8, :]
                        if ci == 0:
                            nc.scalar.copy(out=dsb4[ci][32 * h:32 * h + 1, :],
                                           in_=pn[64:65, :])
                            nc.vector.tensor_copy(out=nsl, in_=pn[0:64, :])
                        else:
                            nc.vector.tensor_copy(
                                out=dsb4[ci][32 * h:32 * h + 1, :],
                                in_=pn[64:65, :])
                            nc.scalar.copy(out=nsl, in_=pn[0:64, :])
                for ci in range(2):
                    t8 = 2 * bp + ci
                    rd4 = wkb.tile([97, 512], F32R, tag="rd4", name="rd4")
                    nc.scalar.activation(
                        out=rd4, in_=dsb4[ci], func=AF.Ln,
                        bias=0.0, scale=1.0,
                    )
                    nc.scalar.activation(
                        out=rd4, in_=rd4, func=AF.Exp,
                        bias=0.0, scale=-1.0,
                    )
                    for h in range(4):
                        bc = psN.tile([64, 512], F32, tag="bc", name="bc",
                                      bufs=2)
                        nc.tensor.matmul(
                            bc, lhsT=t_ind4[:, 64 * h:64 * (h + 1)], rhs=rd4,
                            start=True, stop=True,
                        )
                        nsl = t_ns[h // 2][64 * (h % 2):64 * (h % 2) + 64,
                                           t8, :]
                        nc.vector.tensor_tensor(
                            out=nsl, in0=nsl, in1=bc,
                            op=mybir.AluOpType.mult,
                        )

        # ---- phase B2: y projection --------------------------------
        with tc.tile_pool(name="psY", bufs=8, space="PSUM") as psY, \
             tc.tile_pool(name="wky", bufs=4) as wky:
            for m4 in range(4):
                pys = []
                for t8 in range(TOK_B):
                    py = psY.tile([128, 512], F32, tag="py", name="py")
                    pys.append(py)
                for d in range(2):
                    for t8 in range(TOK_B):
                        nc.tensor.matmul(
                            pys[t8],
                            lhsT=t_wy[d][:, 128 * m4:128 * (m4 + 1)],
                            rhs=t_ns[d][:, t8, :],
                            start=(d == 0), stop=(d == 1),
                        )
                for t8 in range(TOK_B):
                    ysb = wky.tile([128, 512], F32, tag="ysb", name="ysb")
                    if t8 % 2 == 0:
                        nc.scalar.copy(out=ysb, in_=pys[t8])
                    else:
                        nc.vector.tensor_copy(out=ysb, in_=pys[t8])
                    nc.sync.dma_start(
                        out=yT[128 * m4:128 * (m4 + 1),
                               512 * t8:512 * (t8 + 1)],
                        in_=ysb,
                    )

    if split:
        _split_waits(nc)
    return nc


_PROGRAM_CACHE = {}


def _get_program(use_bv, use_mask):
    key = (use_bv, use_mask)
    if key not in _PROGRAM_CACHE:
        _PROGRAM_CACHE[key] = build_program(*key)
    return _PROGRAM_CACHE[key]


def make_in_maps(x, key_padding_mask, Wqkv, bqkv, Wout, bout, omega):
    """Shard + lay out the full inputs into 8 per-core input maps."""
    import ml_dtypes

    bf = ml_dtypes.bfloat16
    Wq, Wk, Wv = Wqkv[0:D], Wqkv[D:2 * D], Wqkv[2 * D:3 * D]
    bq, bk_, bv = bqkv[0:D], bqkv[D:2 * D], bqkv[2 * D:3 * D]
    mask = key_padding_mask

    use_bv = bool(np.any(bv != 0))
    use_mask = bool(np.any(mask))

    onesi = np.zeros((128, 2), bf)
    onesi[0:64, 0] = 1.0
    onesi[64:128, 1] = 1.0

    in_maps = []
    for c in range(8):
        b, hg = c // 2, c % 2
        dsl = slice(256 * hg, 256 * (hg + 1))
        heads = [4 * hg + i for i in range(4)]
        wqk_c = np.concatenate([Wq.T[:, dsl], Wk.T[:, dsl]], axis=1)
        womq_c = np.zeros((128, 512), np.float32)
        womk_c = np.zeros((128, 512), np.float32)
        for i, g in enumerate(heads):
            off = 64 * (i % 2)
            womq_c[off:off + 64, 128 * i:128 * (i + 1)] = omega[g].T * INV_DKRT
        for p in range(2):
            womk_c[0:64, 256 * p:256 * p + 128] = \
                omega[heads[2 * p]].T * INV_DKRT
            womk_c[64:128, 256 * p + 128:256 * p + 256] = \
                omega[heads[2 * p + 1]].T * INV_DKRT
        bqk_vec = np.concatenate([bq[dsl], bk_[dsl]])
        ind4 = np.zeros((97, 256), np.float32)
        for i in range(4):
            ind4[32 * i, 64 * i:64 * (i + 1)] = 1.0
        im = {
            "ind4": ind4,
            "xT": np.ascontiguousarray(x[b].T).astype(bf),
            "wqk": np.ascontiguousarray(wqk_c).astype(bf),
            "wv": np.ascontiguousarray(Wv.T[:, dsl]).astype(bf),
            "womq": womq_c.astype(bf),
            "womk": womk_c.astype(bf),
            "wy": np.ascontiguousarray(Wout[:, dsl].T),
            "bqk": np.ascontiguousarray(bqk_vec.reshape(4, 128).T),
            "onesi": onesi,
        }
        if use_bv:
            im["bvb"] = np.ascontiguousarray(
                np.tile(bv[dsl][None, :], (128, 1)).astype(np.float32)
            )
        if use_mask:
            im["valid"] = np.ascontiguousarray(
                (~mask[b]).astype(np.float32).reshape(TOK_CH, 128).T
            )
        in_maps.append(im)
    return in_maps, (use_bv, use_mask)


def gather_output(per_core_yT, bout):
    """Sum head-group partials, add bout, transpose back to (B, N, D)."""
    y = np.empty((B, N, D), np.float32)
    for b in range(B):
        acc = per_core_yT[2 * b] + per_core_yT[2 * b + 1]
        y[b] = acc.T + bout[None, :]
    return y


def kernel(x, key_padding_mask, Wqkv, bqkv, Wout, bout, omega):
    from concourse.bass_utils import run_bass_kernel_spmd

    x = np.asarray(x, np.float32)
    mask = np.asarray(key_padding_mask)
    Wqkv = np.asarray(Wqkv, np.float32)
    bqkv = np.asarray(bqkv, np.float32)
    Wout = np.asarray(Wout, np.float32)
    bout = np.asarray(bout, np.float32)
    omega = np.asarray(omega, np.float32)

    in_maps, flags = make_in_maps(x, mask, Wqkv, bqkv, Wout, bout, omega)
    nc = _get_program(*flags)
    res = run_bass_kernel_spmd(nc, in_maps, list(range(8)))
    return gather_output([r["yT"] for r in res.results], bout)
